# revision 25
# baseline (speedup 1.0000x reference)
"""Grouped-query attention + output projection on 8 trn2 NeuronCores.

Sharding: by SEQUENCE (queries).  Core i owns queries s in [i*256, (i+1)*256)
and computes ALL 32 heads for its slice; K/V (small) and w_out are replicated.
The projection input (all heads' attention outputs for the local queries) is
then entirely local -- NO collective at all, and attention outputs never
round-trip through DRAM.

Everything runs in a transposed layout so no on-device transposes are needed:

  mm1:   scoresT[k, (h2,q)] = kT_tile.T @ qT2          (contraction over D=128)
  exp:   ACT Exp over [128, 1024] PSUM (2 k-tiles), fused 1/sqrt(D) scale
  denom: DVE-accumulate exp tiles over k, then ONE ones[128,1].T @ acc matmul
  mm2:   outT[d, (h2,q)] = v_tile.T @ expT             (accumulated over k)
  norm:  outT * (ones x 1/denom)                       (broadcast via K=1 matmul)
  proj:  out[s_blk, m_blk] = cc_tile.T @ w_tile        (accumulated over j)

All matmul operands are bf16 (halves SBUF/DMA traffic and PE input power vs
fp32r); PSUM accumulation is fp32.  The denominator reduction rides the DVE
(bf16 2x mode) instead of burning 256 tensor-engine matmuls.  Loops are
group-major so each kT/v stationary load serves two head-pair matmuls.
"""

import sys

import numpy as np

S = 2048
H = 32
G = 8
D = 128
HPG = H // G
MODEL = H * D         # 4096
NCORES = 8
SL = S // NCORES      # 256 queries per core
NP = H // 2           # 16 head-pairs per core
NKT = S // 128        # 16 k tiles
NTP = NKT // 2        # 8 k-tile pairs
NJT = MODEL // 128    # 32 j tiles (proj contraction) == heads
NMB = MODEL // 512    # 8 m blocks

_CACHE = {}


def _build_bass():
    if "/opt/trn_rl_repo" not in sys.path:
        sys.path.insert(0, "/opt/trn_rl_repo")
    import concourse.bacc as bacc
    import concourse.mybir as mybir
    import concourse.tile as tile

    f32 = mybir.dt.float32
    bf16 = mybir.dt.bfloat16
    EXP = mybir.ActivationFunctionType.Exp
    COPY = mybir.ActivationFunctionType.Copy
    scale = float(D) ** -0.5

    nc = bacc.Bacc(None, num_devices=NCORES)
    # q2[p, d, j*256+q] = query[s0+q, 2p+j, d]   (per-core)
    q2 = nc.dram_tensor("q2", [NP, D, 2 * SL], bf16, kind="ExternalInput")
    # kT[g, d, k] = key[k, g, d]                 (replicated)
    kT = nc.dram_tensor("kT", [G, D, S], bf16, kind="ExternalInput")
    # vt[g, kk, t*128+dd] = value[t*128+kk, g, dd]
    vt = nc.dram_tensor("vt", [G, 128, S], bf16, kind="ExternalInput")
    # wt[mb, jj, a*512+mm] = w_out[mb*512+mm, a*128+jj]  (one slab per m-block)
    wt = nc.dram_tensor("wt", [NMB, 128, NJT * 512], bf16, kind="ExternalInput")
    ones_d = nc.dram_tensor("ones", [128, 128], bf16, kind="ExternalInput")
    out = nc.dram_tensor("out", [SL, MODEL], f32, kind="ExternalOutput")

    lp = nc.allow_low_precision("bf16 matmul operands")
    lp.__enter__()
    with tile.TileContext(nc) as tc:
        with (
            tc.tile_pool(name="const", bufs=1) as constp,
            tc.tile_pool(name="kv", bufs=1) as kvp,
            tc.tile_pool(name="qt", bufs=3) as qtp,
            tc.tile_pool(name="expt", bufs=6) as expp,
            tc.tile_pool(name="acc", bufs=6) as accp,
            tc.tile_pool(name="cc", bufs=1) as ccp,
            tc.tile_pool(name="w", bufs=2) as wp,
            tc.tile_pool(name="misc", bufs=4) as miscp,
            tc.tile_pool(name="osb", bufs=3) as outp,
            tc.tile_pool(name="ps_s", bufs=2, space="PSUM") as ps_s,
            tc.tile_pool(name="ps_o", bufs=2, space="PSUM") as ps_o,
            tc.tile_pool(name="ps_t", bufs=2, space="PSUM") as ps_t,
        ):
            # Resident operands
            ones_sb = constp.tile([128, 128], bf16, name="ones_sb")
            nc.sync.dma_start(ones_sb[:], ones_d[:])
            kT_sb = kvp.tile([128, G * S], bf16, name="kT_sb")
            v_sb = kvp.tile([128, G * S], bf16, name="v_sb")
            # Stage group 0 first so the attention pipeline starts immediately;
            # later groups prefetch inside the loop.
            nc.sync.dma_start(kT_sb[:, 0:S], kT[0])
            nc.sync.dma_start(v_sb[:, 0:S], vt[0])

            cc_tiles = []
            for p in range(NP):
                cc_tiles.append(
                    ccp.tile([128, 2 * SL], bf16, tag=f"cc{p}", name=f"cc{p}")
                )

            # ---- Attention, group-major: group g covers head-pairs (2g, 2g+1)
            for g in range(G):
                kbase = g * S
                if g + 1 < G:
                    gn = g + 1
                    nc.sync.dma_start(kT_sb[:, gn * S : (gn + 1) * S], kT[gn])
                    nc.sync.dma_start(v_sb[:, gn * S : (gn + 1) * S], vt[gn])
                q_sbs = []
                psums_o = []
                accs = []
                for hp in range(2):
                    p = 2 * g + hp
                    q_sb = qtp.tile([128, 2 * SL], bf16, tag="q", name="q_sb")
                    nc.sync.dma_start(q_sb[:], q2[p])
                    q_sbs.append(q_sb)
                    psums_o.append(
                        ps_o.tile([128, 2 * SL], f32, tag="o", name="psum_o")
                    )
                    accs.append(None)
                exs = [None, None]
                for tp in range(NTP):
                    t0 = 2 * tp
                    t1 = 2 * tp + 1
                    pss = []
                    for hp in range(2):
                        pss.append(
                            ps_s.tile([128, 1024], f32, tag="scores", name="ps")
                        )
                    # two mm1 per stationary kT tile
                    for j, t in ((0, t0), (1, t1)):
                        for hp in range(2):
                            nc.tensor.matmul(
                                pss[hp][:, j * 512 : (j + 1) * 512],
                                kT_sb[:, kbase + t * 128 : kbase + t * 128 + 128],
                                q_sbs[hp][:],
                                start=True,
                                stop=True,
                            )
                    for hp in range(2):
                        ex = expp.tile([128, 1024], bf16, tag="exp", name="ex")
                        nc.scalar.activation(ex[:], pss[hp][:], EXP, scale=scale)
                        exs[hp] = ex
                    # two mm2 per stationary v tile
                    for j, t in ((0, t0), (1, t1)):
                        for hp in range(2):
                            nc.tensor.matmul(
                                psums_o[hp][:],
                                v_sb[:, kbase + t * 128 : kbase + t * 128 + 128],
                                exs[hp][:, j * 512 : (j + 1) * 512],
                                start=(t == 0),
                                stop=(t == NKT - 1),
                            )
                    # DVE: accumulate denominator partials (ping-pong, never
                    # in-place, so the bf16 2x perf mode can engage)
                    for hp in range(2):
                        a0 = accp.tile([128, 2 * SL], bf16, tag="acc", name="acc")
                        if tp == 0:
                            nc.vector.tensor_add(
                                a0[:], exs[hp][:, 0:512], exs[hp][:, 512:1024]
                            )
                        else:
                            a1 = accp.tile(
                                [128, 2 * SL], bf16, tag="acc", name="acc"
                            )
                            nc.vector.tensor_add(
                                a1[:], accs[hp][:], exs[hp][:, 0:512]
                            )
                            nc.vector.tensor_add(
                                a0[:], a1[:], exs[hp][:, 512:1024]
                            )
                        accs[hp] = a0
                # normalize: denom = ones.T @ acc; recip; broadcast; cc = o * rb
                for hp in range(2):
                    p = 2 * g + hp
                    # den and rb share one PSUM bank: den lands in partition 0,
                    # is consumed by recip, then the broadcast overwrites the
                    # whole tile (Tile serializes the WAR on partition 0).
                    tail = ps_t.tile([128, 2 * SL], f32, tag="tail", name="tail")
                    nc.tensor.matmul(
                        tail[0:1, :], ones_sb[:, 0:1], accs[hp][:], start=True, stop=True
                    )
                    recip = miscp.tile([1, 2 * SL], bf16, tag="recip", name="recip")
                    nc.vector.reciprocal(recip[:], tail[0:1, :])
                    nc.tensor.matmul(
                        tail[:], ones_sb[0:1, :], recip[:], start=True, stop=True
                    )
                    rb_sb = miscp.tile([128, 2 * SL], bf16, tag="rb", name="rb_sb")
                    nc.vector.tensor_copy(rb_sb[:], tail[:])
                    nc.vector.tensor_mul(cc_tiles[p][:], psums_o[hp][:], rb_sb[:])

            # ---- Projection: out[si*128+s, mb*512+m] += cc[j, s] * w'[j, m]
            for mb in range(NMB):
                w_sb = wp.tile([128, NJT * 512], bf16, tag="w", name="w_sb")
                nc.sync.dma_start(w_sb[:], wt[mb])
                pp = ps_s.tile([128, 1024], f32, tag="scores", name="pp")
                for a in range(NJT):
                    for si in range(2):
                        lhs = cc_tiles[a // 2][
                            :, (a % 2) * SL + si * 128 : (a % 2) * SL + si * 128 + 128
                        ]
                        nc.tensor.matmul(
                            pp[:, si * 512 : (si + 1) * 512],
                            lhs,
                            w_sb[:, a * 512 : (a + 1) * 512],
                            start=(a == 0),
                            stop=(a == NJT - 1),
                        )
                for si in range(2):
                    o_sb = outp.tile([128, 512], f32, tag="o", name="o_sb")
                    nc.scalar.activation(o_sb[:], pp[:, si * 512 : (si + 1) * 512], COPY)
                    nc.sync.dma_start(
                        out[si * 128 : si * 128 + 128, mb * 512 : (mb + 1) * 512],
                        o_sb[:],
                    )
    lp.__exit__(None, None, None)
    nc.finalize()
    return nc


def _get_nc():
    if "nc" not in _CACHE:
        _CACHE["nc"] = _build_bass()
    return _CACHE["nc"]


def _make_in_maps(query, key, value, w_out):
    import ml_dtypes

    bf = ml_dtypes.bfloat16
    query = np.asarray(query, dtype=np.float32)
    key = np.asarray(key, dtype=np.float32)
    value = np.asarray(value, dtype=np.float32)
    w_out = np.asarray(w_out, dtype=np.float32)

    # Replicated tensors
    kT = np.ascontiguousarray(key.transpose(1, 2, 0)).astype(bf)  # [G, D, S]
    vt = np.ascontiguousarray(
        value.reshape(NKT, 128, G, D).transpose(2, 1, 0, 3).reshape(G, 128, S)
    ).astype(bf)
    wt = np.ascontiguousarray(
        w_out.reshape(NMB, 512, NJT, 128)
        .transpose(0, 3, 2, 1)
        .reshape(NMB, 128, NJT * 512)
    ).astype(bf)  # [mb, jj, a*512+mm]
    ones = np.ones((128, 128), dtype=bf)

    in_maps = []
    for i in range(NCORES):
        qs = query[i * SL : (i + 1) * SL]  # [SL, H, D]
        # [H, D, SL] -> [NP, 2, D, SL] -> [NP, D, 2, SL] -> [NP, D, 2*SL]
        q2 = (
            np.ascontiguousarray(
                qs.transpose(1, 2, 0)
                .reshape(NP, 2, D, SL)
                .transpose(0, 2, 1, 3)
                .reshape(NP, D, 2 * SL)
            )
        ).astype(bf)
        in_maps.append({"q2": q2, "kT": kT, "vt": vt, "wt": wt, "ones": ones})
    return in_maps


def run_sharded(query, key, value, w_out, trace=False, tmpdir=None):
    """Run the SPMD kernel; returns (out_full [S, MODEL], BassKernelResults)."""
    if "/opt/trn_rl_repo" not in sys.path:
        sys.path.insert(0, "/opt/trn_rl_repo")
    from concourse.bass_utils import run_bass_kernel_spmd

    nc = _get_nc()
    in_maps = _make_in_maps(query, key, value, w_out)
    res = run_bass_kernel_spmd(
        nc, in_maps, list(range(NCORES)), trace=trace, tmpdir=tmpdir
    )
    outs = [np.asarray(res.results[i]["out"]) for i in range(NCORES)]
    full = np.concatenate(outs, axis=0)  # [S, MODEL]
    return full, res


def kernel(query, key, value, mask, w_out, b_out):
    full, _ = run_sharded(query, key, value, w_out, trace=False)
    full = full + np.asarray(b_out, dtype=np.float32)[None, :]
    return full.reshape(S, H, D).astype(np.float32)


# revision 27
# speedup vs baseline: 1.0242x; 1.0242x over previous
"""Grouped-query attention + output projection on 8 trn2 NeuronCores.

Sharding: by SEQUENCE (queries).  Core i owns queries s in [i*256, (i+1)*256)
and computes ALL 32 heads for its slice; K/V (small) and w_out are replicated.
The projection input (all heads' attention outputs for the local queries) is
then entirely local -- NO collective at all, and attention outputs never
round-trip through DRAM.

Everything runs in a transposed layout so no on-device transposes are needed:

  mm1:   scoresT[k, (h2,q)] = kT_tile.T @ qT2          (contraction over D=128)
  exp:   ACT Exp over [128, 1024] PSUM (2 k-tiles), fused 1/sqrt(D) scale
  denom: DVE-accumulate exp tiles over k, then ONE ones[128,1].T @ acc matmul
  mm2:   outT[d, (h2,q)] = v_tile.T @ expT             (accumulated over k)
  norm:  outT * (ones x 1/denom)                       (broadcast via K=1 matmul)
  proj:  out[s_blk, m_blk] = cc_tile.T @ w_tile        (accumulated over j)

All matmul operands are bf16 (halves SBUF/DMA traffic and PE input power vs
fp32r); PSUM accumulation is fp32.  The denominator reduction rides the DVE
(bf16 2x mode) instead of burning 256 tensor-engine matmuls.  Loops are
group-major so each kT/v stationary load serves two head-pair matmuls.
"""

import sys

import numpy as np

S = 2048
H = 32
G = 8
D = 128
HPG = H // G
MODEL = H * D         # 4096
NCORES = 8
SL = S // NCORES      # 256 queries per core
NP = H // 2           # 16 head-pairs per core
NKT = S // 128        # 16 k tiles
NTP = NKT // 2        # 8 k-tile pairs
NJT = MODEL // 128    # 32 j tiles (proj contraction) == heads
NMB = MODEL // 512    # 8 m blocks

_CACHE = {}


def _build_bass():
    if "/opt/trn_rl_repo" not in sys.path:
        sys.path.insert(0, "/opt/trn_rl_repo")
    import concourse.bacc as bacc
    import concourse.mybir as mybir
    import concourse.tile as tile

    f32 = mybir.dt.float32
    bf16 = mybir.dt.bfloat16
    EXP = mybir.ActivationFunctionType.Exp
    COPY = mybir.ActivationFunctionType.Copy
    scale = float(D) ** -0.5

    nc = bacc.Bacc(None, num_devices=NCORES)
    # q2[p, d, j*256+q] = query[s0+q, 2p+j, d]   (per-core)
    q2 = nc.dram_tensor("q2", [NP, D, 2 * SL], bf16, kind="ExternalInput")
    # kT[g, d, k] = key[k, g, d]                 (replicated)
    kT = nc.dram_tensor("kT", [G, D, S], bf16, kind="ExternalInput")
    # vt[g, kk, t*128+dd] = value[t*128+kk, g, dd]
    vt = nc.dram_tensor("vt", [G, 128, S], bf16, kind="ExternalInput")
    # wt[mb, jj, a*512+mm] = w_out[mb*512+mm, a*128+jj]  (one slab per m-block)
    wt = nc.dram_tensor("wt", [NMB, 128, NJT * 512], bf16, kind="ExternalInput")
    ones_d = nc.dram_tensor("ones", [128, 128], bf16, kind="ExternalInput")
    out = nc.dram_tensor("out", [SL, MODEL], f32, kind="ExternalOutput")

    lp = nc.allow_low_precision("bf16 matmul operands")
    lp.__enter__()
    with tile.TileContext(nc) as tc:
        with (
            tc.tile_pool(name="const", bufs=1) as constp,
            tc.tile_pool(name="kv", bufs=1) as kvp,
            tc.tile_pool(name="qt", bufs=3) as qtp,
            tc.tile_pool(name="expt", bufs=6) as expp,
            tc.tile_pool(name="acc", bufs=6) as accp,
            tc.tile_pool(name="cc", bufs=1) as ccp,
            tc.tile_pool(name="w", bufs=2) as wp,
            tc.tile_pool(name="misc", bufs=4) as miscp,
            tc.tile_pool(name="osb", bufs=3) as outp,
            tc.tile_pool(name="ps_s", bufs=2, space="PSUM") as ps_s,
            tc.tile_pool(name="ps_o", bufs=2, space="PSUM") as ps_o,
            tc.tile_pool(name="ps_t", bufs=2, space="PSUM") as ps_t,
        ):
            # Resident operands.  Group 0's K/V stages first so the attention
            # pipeline starts immediately; later groups prefetch in the loop.
            kT_sb = kvp.tile([128, G * S], bf16, name="kT_sb")
            v_sb = kvp.tile([128, G * S], bf16, name="v_sb")
            nc.sync.dma_start(kT_sb[:, 0:S], kT[0])
            ones_sb = constp.tile([128, 128], bf16, name="ones_sb")
            nc.sync.dma_start(v_sb[:, 0:S], vt[0])
            nc.sync.dma_start(ones_sb[:], ones_d[:])

            cc_tiles = []
            for p in range(NP):
                cc_tiles.append(
                    ccp.tile([128, 2 * SL], bf16, tag=f"cc{p}", name=f"cc{p}")
                )

            # ---- Attention, group-major: group g covers head-pairs (2g, 2g+1)
            for g in range(G):
                kbase = g * S
                q_sbs = []
                psums_o = []
                accs = []
                for hp in range(2):
                    p = 2 * g + hp
                    q_sb = qtp.tile([128, 2 * SL], bf16, tag="q", name="q_sb")
                    nc.sync.dma_start(q_sb[:], q2[p])
                    q_sbs.append(q_sb)
                if g + 1 < G:
                    gn = g + 1
                    nc.sync.dma_start(kT_sb[:, gn * S : (gn + 1) * S], kT[gn])
                    nc.sync.dma_start(v_sb[:, gn * S : (gn + 1) * S], vt[gn])
                for hp in range(2):
                    psums_o.append(
                        ps_o.tile([128, 2 * SL], f32, tag="o", name="psum_o")
                    )
                    accs.append(None)
                exs = [None, None]
                for tp in range(NTP):
                    t0 = 2 * tp
                    t1 = 2 * tp + 1
                    pss = []
                    for hp in range(2):
                        pss.append(
                            ps_s.tile([128, 1024], f32, tag="scores", name="ps")
                        )
                    # two mm1 per stationary kT tile
                    for j, t in ((0, t0), (1, t1)):
                        for hp in range(2):
                            nc.tensor.matmul(
                                pss[hp][:, j * 512 : (j + 1) * 512],
                                kT_sb[:, kbase + t * 128 : kbase + t * 128 + 128],
                                q_sbs[hp][:],
                                start=True,
                                stop=True,
                            )
                    for hp in range(2):
                        ex = expp.tile([128, 1024], bf16, tag="exp", name="ex")
                        nc.scalar.activation(ex[:], pss[hp][:], EXP, scale=scale)
                        exs[hp] = ex
                    # two mm2 per stationary v tile
                    for j, t in ((0, t0), (1, t1)):
                        for hp in range(2):
                            nc.tensor.matmul(
                                psums_o[hp][:],
                                v_sb[:, kbase + t * 128 : kbase + t * 128 + 128],
                                exs[hp][:, j * 512 : (j + 1) * 512],
                                start=(t == 0),
                                stop=(t == NKT - 1),
                            )
                    # DVE: accumulate denominator partials (ping-pong, never
                    # in-place, so the bf16 2x perf mode can engage)
                    for hp in range(2):
                        a0 = accp.tile([128, 2 * SL], bf16, tag="acc", name="acc")
                        if tp == 0:
                            nc.vector.tensor_add(
                                a0[:], exs[hp][:, 0:512], exs[hp][:, 512:1024]
                            )
                        else:
                            a1 = accp.tile(
                                [128, 2 * SL], bf16, tag="acc", name="acc"
                            )
                            nc.vector.tensor_add(
                                a1[:], accs[hp][:], exs[hp][:, 0:512]
                            )
                            nc.vector.tensor_add(
                                a0[:], a1[:], exs[hp][:, 512:1024]
                            )
                        accs[hp] = a0
                # normalize: denom = ones.T @ acc; recip; broadcast; cc = o * rb
                for hp in range(2):
                    p = 2 * g + hp
                    # den and rb share one PSUM bank: den lands in partition 0,
                    # is consumed by recip, then the broadcast overwrites the
                    # whole tile (Tile serializes the WAR on partition 0).
                    tail = ps_t.tile([128, 2 * SL], f32, tag="tail", name="tail")
                    nc.tensor.matmul(
                        tail[0:1, :], ones_sb[:, 0:1], accs[hp][:], start=True, stop=True
                    )
                    recip = miscp.tile([1, 2 * SL], bf16, tag="recip", name="recip")
                    nc.vector.reciprocal(recip[:], tail[0:1, :])
                    nc.tensor.matmul(
                        tail[:], ones_sb[0:1, :], recip[:], start=True, stop=True
                    )
                    rb_sb = miscp.tile([128, 2 * SL], bf16, tag="rb", name="rb_sb")
                    nc.vector.tensor_copy(rb_sb[:], tail[:])
                    nc.vector.tensor_mul(cc_tiles[p][:], psums_o[hp][:], rb_sb[:])

            # ---- Projection: out[si*128+s, mb*512+m] += cc[j, s] * w'[j, m]
            for mb in range(NMB):
                w_sb = wp.tile([128, NJT * 512], bf16, tag="w", name="w_sb")
                nc.sync.dma_start(w_sb[:], wt[mb])
                pp = ps_s.tile([128, 1024], f32, tag="scores", name="pp")
                for a in range(NJT):
                    for si in range(2):
                        lhs = cc_tiles[a // 2][
                            :, (a % 2) * SL + si * 128 : (a % 2) * SL + si * 128 + 128
                        ]
                        nc.tensor.matmul(
                            pp[:, si * 512 : (si + 1) * 512],
                            lhs,
                            w_sb[:, a * 512 : (a + 1) * 512],
                            start=(a == 0),
                            stop=(a == NJT - 1),
                        )
                for si in range(2):
                    o_sb = outp.tile([128, 512], f32, tag="o", name="o_sb")
                    nc.scalar.activation(o_sb[:], pp[:, si * 512 : (si + 1) * 512], COPY)
                    nc.sync.dma_start(
                        out[si * 128 : si * 128 + 128, mb * 512 : (mb + 1) * 512],
                        o_sb[:],
                    )
    lp.__exit__(None, None, None)
    nc.finalize()
    return nc


def _get_nc():
    if "nc" not in _CACHE:
        _CACHE["nc"] = _build_bass()
    return _CACHE["nc"]


def _make_in_maps(query, key, value, w_out):
    import ml_dtypes

    bf = ml_dtypes.bfloat16
    query = np.asarray(query, dtype=np.float32)
    key = np.asarray(key, dtype=np.float32)
    value = np.asarray(value, dtype=np.float32)
    w_out = np.asarray(w_out, dtype=np.float32)

    # Replicated tensors
    kT = np.ascontiguousarray(key.transpose(1, 2, 0)).astype(bf)  # [G, D, S]
    vt = np.ascontiguousarray(
        value.reshape(NKT, 128, G, D).transpose(2, 1, 0, 3).reshape(G, 128, S)
    ).astype(bf)
    wt = np.ascontiguousarray(
        w_out.reshape(NMB, 512, NJT, 128)
        .transpose(0, 3, 2, 1)
        .reshape(NMB, 128, NJT * 512)
    ).astype(bf)  # [mb, jj, a*512+mm]
    ones = np.ones((128, 128), dtype=bf)

    in_maps = []
    for i in range(NCORES):
        qs = query[i * SL : (i + 1) * SL]  # [SL, H, D]
        # [H, D, SL] -> [NP, 2, D, SL] -> [NP, D, 2, SL] -> [NP, D, 2*SL]
        q2 = (
            np.ascontiguousarray(
                qs.transpose(1, 2, 0)
                .reshape(NP, 2, D, SL)
                .transpose(0, 2, 1, 3)
                .reshape(NP, D, 2 * SL)
            )
        ).astype(bf)
        in_maps.append({"q2": q2, "kT": kT, "vt": vt, "wt": wt, "ones": ones})
    return in_maps


def run_sharded(query, key, value, w_out, trace=False, tmpdir=None):
    """Run the SPMD kernel; returns (out_full [S, MODEL], BassKernelResults)."""
    if "/opt/trn_rl_repo" not in sys.path:
        sys.path.insert(0, "/opt/trn_rl_repo")
    from concourse.bass_utils import run_bass_kernel_spmd

    nc = _get_nc()
    in_maps = _make_in_maps(query, key, value, w_out)
    res = run_bass_kernel_spmd(
        nc, in_maps, list(range(NCORES)), trace=trace, tmpdir=tmpdir
    )
    outs = [np.asarray(res.results[i]["out"]) for i in range(NCORES)]
    full = np.concatenate(outs, axis=0)  # [S, MODEL]
    return full, res


def kernel(query, key, value, mask, w_out, b_out):
    full, _ = run_sharded(query, key, value, w_out, trace=False)
    full = full + np.asarray(b_out, dtype=np.float32)[None, :]
    return full.reshape(S, H, D).astype(np.float32)
